# revision 14
# baseline (speedup 1.0000x reference)
"""Bidirectional chamfer loss kernel for Trainium2 (8 NeuronCores).

Problem (hardcoded): B=2 batches, V1=8192 gt points, V2=8192 pred points, 3D.
  d2[b,i,j] = max(0, |xp_i|^2 + |gt_j|^2 - 2 xp_i.gt_j),  xp = x_pred * mask
  loss_pred2gt[b,i] = sqrt(min_j d2) * 100
  loss_gt2pred[b,j] = sqrt(min_i d2) * 100
  loss_conf = (loss_pred2gt * conf - ln(conf)) * mask ; loss_pred2gt *= mask

Sharding: 8 cores = 2 batches x 4 V2-slices (2048 preds/core vs full 8192 gt).
Each core computes its pred2gt slice exactly, and a partial gt2pred
(min over its 2048 preds); the host combines partials with np.minimum
(sqrt is monotone, so combining after sqrt*100 is exact).

Device kernel (per core, SPMD), "k15" variant:
  PE matmul cost is N moving columns regardless of contraction depth K<=128,
  so the fp16 hi/lo split that needs 3 separate matmuls in the naive form
  (A_hi.G_hi + A_lo.G_hi + A_hi.G_lo) is packed into ONE K=15 matmul:
    lhsT rows  0-4  = A_hi   rhs rows  0-4  = G_hi
    lhsT rows  5-9  = A_lo   rhs rows  5-9  = G_hi
    lhsT rows 10-14 = A_hi   rhs rows 10-14 = G_lo
  with A = [-2xp | -2xp_y | -2xp_z | |xp|^2 | 1], G = [gt | 1 | |gt|^2]
  (the K=5 augmented-operand distance expansion). PSUM accumulates in fp32;
  the dropped A_lo.G_lo term is ~2^-22 relative -- fp32-grade d2 at fp16
  matmul cost.

  Per (pred-tile 128, gt-group 2048): 4 N=512 matmuls -> one PSUM tile;
  ScalarE downconverts it once to fp16 SBUF (this enables the DVE 2x_1P
  perf mode); DVE folds it into a per-group column-min accumulator
  (tensor_tensor min) and a per-(tile,group) row min (tensor_reduce).
  Columns finish with DVE 32x32 transposes + reduces as in the f16 path.

Sync-wait discipline: every instruction has at most one cross-engine
dependency (PSUM tile freed by its single ScalarE reader; s16 freed by its
DVE readers; accumulator init on the DVE itself), which Bacc's compile()
legalizes without extra event semaphores.
"""

import numpy as np

B = 2
V1 = 8192  # gt points
V2 = 8192  # pred points (total)
N_CORES = 8
SLICES = N_CORES // B  # V2-slices per batch
V2C = V2 // SLICES  # pred points per core

_BUILT = {}


def _build(v1, v2c, mm_dtype_name="float32", repeat=1):
    import concourse.tile as tile
    from concourse import bacc, mybir

    f32 = mybir.dt.float32
    mm_dt = getattr(mybir.dt, mm_dtype_name)
    MIN = mybir.AluOpType.min
    MUL = mybir.AluOpType.mult
    SUB = mybir.AluOpType.subtract
    X = mybir.AxisListType.X
    AF = mybir.ActivationFunctionType

    npt = v2c // 128  # pred tiles
    ngc = v1 // 512  # gt chunks (matmul moving dim)
    ngt = v1 // 128  # gt output tiles
    BIG = 3.0e38

    nc = bacc.Bacc()
    ag_in = nc.dram_tensor("ag", [5, v2c + v1], mm_dt, kind="ExternalInput")
    mc_in = nc.dram_tensor("mc", [128, 2 * npt], f32, kind="ExternalInput")
    o_all = nc.dram_tensor("o_all", [128, 2 * npt + ngt], f32, kind="ExternalOutput")

    with tile.TileContext(nc) as tc:
        with (
            tc.tile_pool(name="persist", bufs=1) as P,
            tc.tile_pool(name="rowp", bufs=2) as RP,
            tc.tile_pool(name="small", bufs=1) as SP,
            tc.tile_pool(name="mmps", bufs=6, space="PSUM") as MMPS,
            tc.tile_pool(name="trps", bufs=2, space="PSUM") as TRPS,
        ):
            AG = P.tile([5, v2c + v1], mm_dt, tag="AG")
            A = AG[:, 0:v2c]
            G = AG[:, v2c : v2c + v1]
            MC = P.tile([128, 2 * npt], f32, tag="MC")
            mc_sb = P.tile([128, 2 * npt], f32, tag="mc_sb")
            mask_ep = mc_sb[:, 0:npt]
            conf_ep = mc_sb[:, npt : 2 * npt]
            ident_pool = P.tile([128, 128], f32, tag="identp")
            ident = P.tile([128, 128], f32, tag="ident")
            colacc = [
                P.tile([128, 512], f32, tag=f"col{g}", name=f"col{g}")
                for g in range(ngc)
            ]
            p2g_min = P.tile([128, npt], f32, tag="p2gmin")
            g2p_min = P.tile([128, ngt], f32, tag="g2pmin")

            nc.gpsimd.memset(ident_pool[:], 0.0)
            nc.gpsimd.affine_select(
                out=ident_pool[:],
                in_=ident_pool[:],
                compare_op=mybir.AluOpType.not_equal,
                fill=1.0,
                base=0,
                pattern=[[-1, 128]],
                channel_multiplier=1,
            )
            nc.vector.tensor_copy(ident[:], ident_pool[:])

            nc.sync.dma_start(AG[:], ag_in[:, :])
            nc.sync.dma_start(MC[:], mc_in[:, :])
            nc.vector.tensor_copy(mc_sb[:], MC[:])

            for g in range(ngc):
                nc.vector.memset(colacc[g][:], BIG)

            for pt in [p for _ in range(repeat) for p in range(npt)]:
                rowacc = RP.tile([128, 512], f32, tag="rowacc")
                lhsT = A[:, pt * 128 : (pt + 1) * 128]
                for gc in range(ngc):
                    ps = MMPS.tile([128, 512], f32, tag="mm")
                    nc.tensor.matmul(
                        ps[:],
                        lhsT,
                        G[:, gc * 512 : (gc + 1) * 512],
                        start=True,
                        stop=True,
                    )
                    if gc == 0:
                        nc.vector.tensor_copy(rowacc[:], ps[:])
                    else:
                        nc.vector.tensor_tensor(rowacc[:], rowacc[:], ps[:], op=MIN)
                    nc.vector.tensor_tensor(
                        colacc[gc][:], colacc[gc][:], ps[:], op=MIN
                    )
                nc.vector.tensor_reduce(
                    p2g_min[:, pt : pt + 1], rowacc[:], axis=X, op=MIN
                )

            for gc in range(ngc):
                for q in range(4):
                    tp = TRPS.tile([128, 128], f32, tag="tr")
                    nc.tensor.transpose(
                        tp[:], colacc[gc][:, q * 128 : (q + 1) * 128], ident[:]
                    )
                    j = gc * 4 + q
                    nc.vector.tensor_reduce(
                        g2p_min[:, j : j + 1], tp[:], axis=X, op=MIN
                    )

            out_sb = SP.tile([128, 2 * npt + ngt], f32, tag="out_sb")
            nc.vector.tensor_scalar_max(p2g_min[:], p2g_min[:], 0.0)
            ep = SP.tile([128, npt], f32, tag="ep")
            nc.scalar.activation(ep[:], p2g_min[:], AF.Sqrt, scale=10000.0)
            lnc = SP.tile([128, npt], f32, tag="lnc")
            nc.scalar.activation(lnc[:], conf_ep[:], AF.Ln)
            nc.vector.tensor_tensor(
                out_sb[:, npt : 2 * npt], ep[:], mask_ep[:], op=MUL
            )
            o2 = SP.tile([128, npt], f32, tag="o2")
            nc.vector.tensor_tensor(o2[:], ep[:], conf_ep[:], op=MUL)
            nc.vector.tensor_tensor(o2[:], o2[:], lnc[:], op=SUB)
            nc.vector.tensor_tensor(out_sb[:, 0:npt], o2[:], mask_ep[:], op=MUL)

            nc.vector.tensor_scalar_max(g2p_min[:], g2p_min[:], 0.0)
            g2 = SP.tile([128, ngt], f32, tag="g2")
            nc.scalar.activation(g2[:], g2p_min[:], AF.Sqrt, scale=10000.0)
            nc.vector.tensor_copy(out_sb[:, 2 * npt :], g2[:])
            nc.sync.dma_start(o_all[:, :], out_sb[:])

    nc.compile()
    return nc


def _build_k15(v1, v2c, repeat=1, mmw=512):
    """K=15 packed hi/lo fp16 variant (see module docstring)."""
    import concourse.tile as tile
    from concourse import bacc, mybir

    f32 = mybir.dt.float32
    f16 = mybir.dt.float16
    MIN = mybir.AluOpType.min
    MAX = mybir.AluOpType.max
    MUL = mybir.AluOpType.mult
    SUB = mybir.AluOpType.subtract
    X = mybir.AxisListType.X
    AF = mybir.ActivationFunctionType

    npt = v2c // 128  # pred tiles
    W = min(2048, v1)  # gt group width: one PSUM tile, one ScalarE downconvert
    ng = v1 // W  # gt groups
    gps_g = ng - 1  # column-fold chain owned by GPSIMD (load-balance off DVE)
    ow = 2 * npt  # fused conf/p2g output width
    S = v2c + v1

    nc = bacc.Bacc()
    ag_in = nc.dram_tensor("ag", [15, S], f16, kind="ExternalInput")
    mc_in = nc.dram_tensor("mc", [128, 2 * npt], f32, kind="ExternalInput")
    o_all = nc.dram_tensor("o_all", [128, ow], f32, kind="ExternalOutput")
    g2p_out = nc.dram_tensor("g2p", [1, v1], f32, kind="ExternalOutput")

    with tile.TileContext(nc) as tc:
        with (
            tc.tile_pool(name="persist", bufs=1) as P,
            tc.tile_pool(name="s16p", bufs=3) as S16P,
            tc.tile_pool(name="rowp", bufs=2) as RP,
            tc.tile_pool(name="hp", bufs=2) as HP,
            tc.tile_pool(name="small", bufs=1) as SP,
            tc.tile_pool(name="trp", bufs=2) as TRP,
            tc.tile_pool(name="mmps", bufs=2, space="PSUM") as MMPS,
        ):
            AG = P.tile([15, S], f16, tag="AG")
            A = AG[:, 0:v2c]
            G = AG[:, v2c:S]
            MC = P.tile([128, 2 * npt], f32, tag="MC")
            mc_sb = P.tile([128, 2 * npt], f32, tag="mc_sb")
            mask_ep = mc_sb[:, 0:npt]
            conf_ep = mc_sb[:, npt : 2 * npt]
            colacc = [
                P.tile([128, W], f16, tag=f"col{g}", name=f"col{g}")
                for g in range(ng)
            ]
            p2g_min = P.tile([128, npt], f32, tag="p2gmin")
            g2ps = P.tile([1, v1], f32, tag="g2ps")

            nc.sync.dma_start(AG[:], ag_in[:, :])
            nc.sync.dma_start(MC[:], mc_in[:, :])
            nc.vector.tensor_copy(mc_sb[:], MC[:])

            # ---- main loop ----
            # Row path avoids the 1x-mode TensorReduce on the hot [128, W]
            # tiles: a TT min of the tile's two halves (2x_1P, both read
            # ports packed -> 4 elem/cycle) + a TT fold into rowacc; only a
            # W/2-wide reduce per pred tile remains at 1x. Col accumulators
            # are seeded by a 4x-mode copy at pt==0 (no memset, no fold).
            H = W // 2
            for pt in [p for _ in range(repeat) for p in range(npt)]:
                lhsT = A[:, pt * 128 : (pt + 1) * 128]
                rowacc = RP.tile([128, H], f16, tag="rowacc")
                for g in range(ng):
                    ps = MMPS.tile([128, W], f32, tag="mm")
                    for i in range(W // mmw):
                        nc.tensor.matmul(
                            ps[:, i * mmw : (i + 1) * mmw],
                            lhsT,
                            G[:, g * W + i * mmw : g * W + (i + 1) * mmw],
                            start=True,
                            stop=True,
                        )
                    s16 = S16P.tile([128, W], f16, tag="s16")
                    nc.scalar.copy(s16[:], ps[:])
                    if pt == 0:
                        nc.vector.tensor_copy(colacc[g][:], s16[:])
                    else:
                        nc.vector.tensor_tensor(
                            colacc[g][:], colacc[g][:], s16[:], op=MAX
                        )
                    if g == 0:
                        nc.vector.tensor_tensor(
                            rowacc[:], s16[:, 0:H], s16[:, H:W], op=MAX
                        )
                    else:
                        h = HP.tile([128, H], f16, tag="h")
                        nc.vector.tensor_tensor(
                            h[:], s16[:, 0:H], s16[:, H:W], op=MAX
                        )
                        nc.vector.tensor_tensor(rowacc[:], rowacc[:], h[:], op=MAX)
                nc.vector.tensor_reduce(
                    p2g_min[:, pt : pt + 1], rowacc[:], axis=X, op=MAX
                )

            # ---- column (gt2pred) finish: GPSIMD partition-axis min ----
            for g in range(ng):
                nc.gpsimd.tensor_reduce(
                    g2ps[:, g * W : (g + 1) * W],
                    colacc[g][:],
                    axis=mybir.AxisListType.C,
                    op=MAX,
                )

            # ---- epilogue ----
            out_sb = SP.tile([128, ow], f32, tag="out_sb")
            nc.vector.tensor_scalar_min(p2g_min[:], p2g_min[:], 0.0)
            ep = SP.tile([128, npt], f32, tag="ep")
            # sqrt(10000*x) == 100*sqrt(x)
            nc.scalar.activation(ep[:], p2g_min[:], AF.Sqrt, scale=-10000.0)
            lnc = SP.tile([128, npt], f32, tag="lnc")
            nc.scalar.activation(lnc[:], conf_ep[:], AF.Ln)
            nc.vector.tensor_tensor(
                out_sb[:, npt : 2 * npt], ep[:], mask_ep[:], op=MUL
            )
            o2 = SP.tile([128, npt], f32, tag="o2")
            nc.vector.tensor_tensor(o2[:], ep[:], conf_ep[:], op=MUL)
            nc.vector.tensor_tensor(o2[:], o2[:], lnc[:], op=SUB)
            nc.vector.tensor_tensor(out_sb[:, 0:npt], o2[:], mask_ep[:], op=MUL)

            nc.vector.tensor_scalar_min(g2ps[:], g2ps[:], 0.0)
            g2 = SP.tile([1, v1], f32, tag="g2")
            nc.scalar.activation(g2[:], g2ps[:], AF.Sqrt, scale=-10000.0)
            nc.sync.dma_start(o_all[:, :], out_sb[:])
            nc.sync.dma_start(g2p_out[:, :], g2[:])

    nc.compile()
    return nc


def get_nc(v1=V1, v2c=V2C, mm_dtype_name="float32", repeat=1, variant="k15"):
    key = (v1, v2c, mm_dtype_name, repeat, variant)
    if key not in _BUILT:
        if variant == "k15":
            _BUILT[key] = _build_k15(v1, v2c, repeat)
        else:
            _BUILT[key] = _build(v1, v2c, mm_dtype_name, repeat)
    return _BUILT[key]


def make_aug(gt, xp):
    """Fused augmented matmul operand [A | G]: one K=5 matmul yields the
    full squared-distance expansion |xp|^2 + |gt|^2 - 2 xp.gt."""
    v2c = xp.shape[0]
    v1 = gt.shape[0]
    ag = np.empty((5, v2c + v1), np.float32)
    ag[0:3, :v2c] = -2.0 * xp.T
    ag[3, :v2c] = (xp * xp).sum(-1)
    ag[4, :v2c] = 1.0
    ag[0:3, v2c:] = gt.T
    ag[3, v2c:] = 1.0
    ag[4, v2c:] = (gt * gt).sum(-1)
    return ag


def make_aug15(gt, xp):
    """K=15 packed hi/lo fp16 operand: rows 0-4 hi.hi, 5-9 A_lo vs G_hi,
    10-14 A_hi vs G_lo (the lo.lo term is dropped, ~2^-22 relative)."""
    v2c = xp.shape[0]
    ag = make_aug(gt, xp)
    ag[:, :v2c] *= -1.0  # negated A side -> matmul yields -d2 (max-fold scheme)
    hi = ag.astype(np.float16)
    lo = (ag - hi.astype(np.float32)).astype(np.float16)
    ag15 = np.empty((15, ag.shape[1]), np.float16)
    ag15[0:5] = hi
    ag15[5:10, :v2c] = lo[:, :v2c]
    ag15[5:10, v2c:] = hi[:, v2c:]
    ag15[10:15, :v2c] = hi[:, :v2c]
    ag15[10:15, v2c:] = lo[:, v2c:]
    return ag15


def make_in_maps(x_gt, x_pred, mask, confidence, variant="k15"):
    """Shard full inputs into per-core input maps (host-side layout only)."""
    npt = V2C // 128
    in_maps = []
    for c in range(N_CORES):
        b, s = divmod(c, SLICES)
        sl = slice(s * V2C, (s + 1) * V2C)
        xp = x_pred[b, sl] * mask[b, sl, None]  # (V2C, 3) masked preds
        m = mask[b, sl]
        cf = confidence[b, sl]
        if variant == "k15":
            ag = make_aug15(x_gt[b], xp)
        else:
            ag = make_aug(x_gt[b], xp)
        mc = np.empty((128, 2 * npt), np.float32)
        mc[:, :npt] = m.reshape(npt, 128).T
        mc[:, npt:] = cf.reshape(npt, 128).T
        in_maps.append({"ag": ag, "mc": mc})
    return in_maps


def assemble_outputs(results):
    """Gather per-core outputs back to full shapes."""
    loss_conf = np.empty((B, V2), dtype=np.float32)
    loss_p2g = np.empty((B, V2), dtype=np.float32)
    loss_g2p = np.full((B, V1), np.inf, dtype=np.float32)
    for c in range(N_CORES):
        b, s = divmod(c, SLICES)
        sl = slice(s * V2C, (s + 1) * V2C)
        npt = V2C // 128
        o = results[c]["o_all"]
        loss_conf[b, sl] = o[:, 0:npt].T.reshape(V2C)
        loss_p2g[b, sl] = o[:, npt : 2 * npt].T.reshape(V2C)
        if "g2p" in results[c]:
            part = results[c]["g2p"].reshape(V1)  # k15: natural gt order
        else:
            part = o[:, 2 * npt :].T.reshape(V1)  # f32 variant: [p, gtile]
        np.minimum(loss_g2p[b], part, out=loss_g2p[b])
    return loss_conf, loss_p2g, loss_g2p


def kernel(x_gt, x_pred, mask, confidence):
    from concourse.bass_utils import run_bass_kernel_spmd

    nc = get_nc()
    in_maps = make_in_maps(
        np.asarray(x_gt), np.asarray(x_pred), np.asarray(mask), np.asarray(confidence)
    )
    res = run_bass_kernel_spmd(nc, in_maps, list(range(N_CORES)))
    return assemble_outputs(res.results)
